# revision 11
# baseline (speedup 1.0000x reference)
"""Trainium2 Bass kernel for nn_Attention4D (dense_transformer).

Strategy: pure data parallel over batch B=256 -> 32 samples/core on 8 cores.
Per core, samples are processed in pairs (matmul free dims >= 256 keep fp32r
matmuls at full rate). BatchNorms are folded into conv weights/biases on the
host. All matmuls run in float32r; the depthwise 3x3 conv runs on the tensor
engine as 9 diagonal-matmul accumulation passes into PSUM using a 16x16
zero-padded spatial image (so each shift is a flat 1D offset and every AP is
3-dim). The attention-output matmuls accumulate into the same PSUM tile, so
relu(attn_out + v_local) + bias falls out of a single PSUM evacuation.
Spatial positions use the padded 16x16 layout (n = 16*(y+1) + (x+1)) from the
attention rhs onward; pad positions carry garbage and are never evacuated.
"""
import sys

sys.path.insert(0, "/opt/trn_rl_repo")

import numpy as np
import concourse.bass as bass
import concourse.mybir as mybir
import concourse.tile as tile
from concourse.bass_utils import run_bass_kernel_spmd

F32 = mybir.dt.float32
F32R = mybir.dt.float32r
U32 = mybir.dt.uint32

B, C, H = 256, 384, 14
NH, KD = 8, 32
D = 128
DH = 1024
N = H * H              # 196
NP = 256               # padded 16x16 image
SCALE = KD ** -0.5
EPS = 1e-5
NCORES = 8
S = B // NCORES        # 32 samples per core
NPAIR = S // 2
M0 = 98                # attention m chunk (196 = 2*98)


def _split_multi_waits(nc):
    """walrus here allows ONE sync wait per instruction; split extras into
    same-engine NoOps placed immediately before (engine queues are FIFO)."""
    n_split = 0
    for f in nc.m.functions:
        for bb in f.blocks:
            insts = list(bb.instructions)
            out = []
            changed = False
            for inst in insts:
                si = inst.sync_info
                if si is not None and len(si.on_wait) > 1:
                    waits = list(si.on_wait)
                    ups = list(si.on_update)
                    for j, w in enumerate(waits[:-1]):
                        nop = mybir.InstNoOp(
                            name=f"{inst.name}-ws{j}", engine=inst.engine,
                            ins=[], outs=[],
                        )
                        nop.sync_info = mybir.SyncInfo(on_wait=[w], on_update=[])
                        out.append(nop)
                        n_split += 1
                    inst.sync_info = mybir.SyncInfo(on_wait=[waits[-1]], on_update=ups)
                    changed = True
                out.append(inst)
            if changed:
                try:
                    bb.instructions[:] = out
                except TypeError:
                    bb.instructions = out
    return n_split


def build_nc(repeat=1):
    nc = bass.Bass()

    xin = nc.declare_dram_parameter("xin", [C, S, N], F32R, isOutput=False)
    wqk = nc.declare_dram_parameter("wqk", [C, 512], F32R, isOutput=False)
    wv = nc.declare_dram_parameter("wv", [C, DH], F32R, isOutput=False)
    wp = nc.declare_dram_parameter("wp", [DH, C], F32R, isOutput=False)
    dgd = nc.declare_dram_parameter("dgd", [128, 72, 128], F32R, isOutput=False)
    bqk_d = nc.declare_dram_parameter("bqk", [128, 4], F32, isOutput=False)
    bv_d = nc.declare_dram_parameter("bv", [128, 8], F32, isOutput=False)
    bvrow_d = nc.declare_dram_parameter("bvrow", [1, DH], F32R, isOutput=False)
    bvl_d = nc.declare_dram_parameter("bvl", [128, 8], F32, isOutput=False)
    bp_d = nc.declare_dram_parameter("bp", [1, C], F32R, isOutput=False)
    selq_d = nc.declare_dram_parameter("selq", [8, 256], F32R, isOutput=False)
    bo2_d = nc.declare_dram_parameter("bo2", [128, 32], F32R, isOutput=False)
    ones1_d = nc.declare_dram_parameter("ones1", [1, 128], F32R, isOutput=False)
    onesn_d = nc.declare_dram_parameter("onesn", [1, 392], F32R, isOutput=False)
    out_d = nc.declare_dram_parameter("out", [C, S, N], F32, isOutput=True)

    with tile.TileContext(nc) as tc:
        nc_lp = nc.allow_low_precision("fp32r pipeline; accumulation stays fp32 in PSUM")
        nc_lp.__enter__()
        with (
            tc.tile_pool(name="wpool", bufs=1) as wpool,
            tc.tile_pool(name="xpool", bufs=2) as xpool,
            tc.tile_pool(name="qkpool", bufs=2) as qkpool,
            tc.tile_pool(name="vtpool", bufs=1) as vtpool,
            tc.tile_pool(name="opool", bufs=1) as opool,
            tc.tile_pool(name="smpool", bufs=2) as smpool,
            tc.tile_pool(name="fixpool", bufs=1) as fixpool,
            tc.tile_pool(name="ps_main", bufs=3, space="PSUM") as ps_main,
            tc.tile_pool(name="ps_at", bufs=3, space="PSUM") as ps_at,
            tc.tile_pool(name="ps_dw", bufs=2, space="PSUM") as ps_dw,
        ):
            # ---- weights / constants ----
            wqk_sb = wpool.tile([128, 3, 512], F32R, name="wqk_sb")
            nc.sync.dma_start(out=wqk_sb[:], in_=wqk.rearrange("(k p) m -> p k m", p=128))
            wv_sb = wpool.tile([128, 3, DH], F32R, name="wv_sb")
            nc.sync.dma_start(out=wv_sb[:], in_=wv.rearrange("(k p) m -> p k m", p=128))
            wp_sb = wpool.tile([128, 8, C], F32R, name="wp_sb")
            nc.sync.dma_start(out=wp_sb[:], in_=wp.rearrange("(k p) m -> p k m", p=128))
            dg_sb = wpool.tile([128, 72, 128], F32R, name="dg_sb")
            nc.sync.dma_start(out=dg_sb[:], in_=dgd[:])
            bqk_sb = wpool.tile([128, 4], F32, name="bqk_sb")
            nc.sync.dma_start(out=bqk_sb[:], in_=bqk_d[:])
            bv_sb = wpool.tile([128, 8], F32, name="bv_sb")
            nc.sync.dma_start(out=bv_sb[:], in_=bv_d[:])
            bvrow_sb = wpool.tile([1, DH], F32R, name="bvrow_sb")
            nc.sync.dma_start(out=bvrow_sb[:], in_=bvrow_d[:])
            bvl_sb = wpool.tile([128, 8], F32, name="bvl_sb")
            nc.sync.dma_start(out=bvl_sb[:], in_=bvl_d[:])
            bp_sb = wpool.tile([1, C], F32R, name="bp_sb")
            nc.sync.dma_start(out=bp_sb[:], in_=bp_d[:])
            selq_sb = wpool.tile([8, 256], F32R, name="selq_sb")
            nc.sync.dma_start(out=selq_sb[:], in_=selq_d[:])
            bo2_sb = wpool.tile([128, 32], F32R, name="bo2_sb")
            nc.sync.dma_start(out=bo2_sb[:], in_=bo2_d[:])
            ones1_sb = wpool.tile([1, 128], F32R, name="ones1_sb")
            nc.sync.dma_start(out=ones1_sb[:], in_=ones1_d[:])
            onesn_sb = wpool.tile([1, 392], F32R, name="onesn_sb")
            nc.sync.dma_start(out=onesn_sb[:], in_=onesn_d[:])

            # fixed tiles: attention staging per head, padded-v per chunk
            attn_sb = []
            for h in range(NH):
                t = fixpool.tile([M0, 2, NP], F32R, name=f"attn_sb{h}")
                attn_sb.append(t)
            v_fix = []
            for c in range(8):
                t = fixpool.tile([128, 2, NP], F32R, name=f"v_fix{c}")
                nc.vector.memset(t.bitcast(U32), 0)  # zero halo, once
                v_fix.append(t)

            out_sb = []

            def pair_body(p):
                # ---- load x pair: 3 tiles [128, 2, 196] ----
                x_sb = []
                for kc in range(3):
                    xt = xpool.tile([128, 2, N], F32R, name="xt", tag=f"x{kc}")
                    nc.sync.dma_start(
                        out=xt[:],
                        in_=xin[kc * 128:(kc + 1) * 128, 2 * p:2 * p + 2, :],
                    )
                    x_sb.append(xt)

                # ---- q,k convs -> psum -> evac(+bias) + abs ----
                qk_sb = []
                abs_sb = []
                for oc in range(4):
                    ps = ps_main.tile([128, 392], F32, name="psqk", tag="m")
                    for kc in range(3):
                        nc.tensor.matmul(
                            ps[:],
                            wqk_sb[:, kc, oc * 128:(oc + 1) * 128],
                            x_sb[kc].rearrange("p s n -> p (s n)"),
                            start=(kc == 0), stop=(kc == 2),
                        )
                    qt = qkpool.tile([128, 392], F32, name="qt", tag=f"qk{oc}")
                    nc.scalar.add(qt[:], ps[:], bqk_sb[:, oc:oc + 1])
                    at = qkpool.tile([128, 392], F32R, name="at", tag=f"ab{oc}")
                    nc.scalar.activation(at[:], qt[:], mybir.ActivationFunctionType.Abs)
                    qk_sb.append(qt)
                    abs_sb.append(at)

                # ---- Sq,Sk -> reciprocal -> rqB/rkB -> q',k' ----
                qn_sb = []  # q' [128, 2, 256] padded-image layout
                kn_sb = []  # k' [128, 2, 196] flat
                for qk in range(2):
                    ssum = ps_at.tile([8, 392], F32, name="ssum", tag="a")
                    for kc in range(2):
                        nc.tensor.matmul(
                            ssum[:],
                            bo2_sb[:, 16 * qk + 8 * kc:16 * qk + 8 * kc + 8],
                            abs_sb[2 * qk + kc][:],
                            start=(kc == 0), stop=(kc == 1),
                        )
                    smax = smpool.tile([8, 392], F32, name="smax", tag="sm")
                    nc.vector.tensor_scalar_max(smax[:], ssum[:], 1e-12 / SCALE)
                    rcp = smpool.tile([8, 392], F32R, name="rcp", tag="rc")
                    nc.vector.reciprocal(rcp[:], smax[:])
                    for kc in range(2):
                        rb = ps_at.tile([128, 392], F32, name="rb", tag="a")
                        nc.tensor.matmul(
                            rb[:], selq_sb[:, kc * 128:(kc + 1) * 128], rcp[:],
                            start=True, stop=True,
                        )
                        if qk == 0:
                            qn = qkpool.tile([128, 2, NP], F32R, name="qn", tag=f"qn{kc}")
                            for s in range(2):
                                nc.vector.tensor_tensor(
                                    qn[:, s, :].rearrange("p (y x) -> p y x", y=16)[:, 1:15, 1:15],
                                    qk_sb[kc][:, s * N:(s + 1) * N].rearrange("p (y x) -> p y x", y=H),
                                    rb[:, s * N:(s + 1) * N].rearrange("p (y x) -> p y x", y=H),
                                    mybir.AluOpType.mult,
                                )
                            qn_sb.append(qn)
                        else:
                            kn = qkpool.tile([128, 2, N], F32R, name="kn", tag=f"kn{kc}")
                            nc.vector.tensor_tensor(
                                kn.rearrange("p s n -> p (s n)"),
                                qk_sb[2 + kc][:], rb[:], mybir.AluOpType.mult,
                            )
                            kn_sb.append(kn)

                # ---- v conv [dh, n] -> evac with bias into padded image ----
                for oc in range(8):
                    ps = ps_main.tile([128, 392], F32, name="psv", tag="m")
                    for kc in range(3):
                        nc.tensor.matmul(
                            ps[:],
                            wv_sb[:, kc, oc * 128:(oc + 1) * 128],
                            x_sb[kc].rearrange("p s n -> p (s n)"),
                            start=(kc == 0), stop=(kc == 2),
                        )
                    for s in range(2):
                        nc.scalar.add(
                            v_fix[oc][:, s, :].rearrange("p (y x) -> p y x", y=16)[:, 1:15, 1:15],
                            ps[:, s * N:(s + 1) * N].rearrange("p (y x) -> p y x", y=H),
                            bv_sb[:, oc:oc + 1],
                        )

                # ---- v_T [m-chunk, dh] per (s, mchunk) ----
                vT_sb = {}
                for s in range(2):
                    for m in range(2):
                        vT = vtpool.tile([M0, DH], F32R, name="vT", tag=f"vT{s}{m}")
                        for half in range(2):
                            ps = ps_main.tile([M0, 512], F32, name="psvt", tag="m")
                            for kc in range(3):
                                nc.tensor.matmul(
                                    ps[:],
                                    x_sb[kc][:, s, m * M0:m * M0 + M0],
                                    wv_sb[:, kc, half * 512:(half + 1) * 512],
                                    start=(kc == 0), stop=False,
                                )
                            nc.tensor.matmul(
                                ps[:], ones1_sb[:, 0:M0],
                                bvrow_sb[:, half * 512:(half + 1) * 512],
                                start=False, stop=True,
                            )
                            if (s + m + half) % 2 == 0:
                                nc.scalar.copy(vT[:, half * 512:(half + 1) * 512], ps[:])
                            else:
                                nc.vector.tensor_copy(vT[:, half * 512:(half + 1) * 512], ps[:])
                        vT_sb[(s, m)] = vT

                # ---- attention + fused dwconv/out per head (c == h) ----
                taps = [(0, 0)] + [
                    (dy, dx) for dy in (-1, 0, 1) for dx in (-1, 0, 1)
                    if not (dy == 0 and dx == 0)
                ]
                for h in range(NH):
                    c = h
                    kc, hh = divmod(h, 4)
                    psd = ps_dw.tile([128, 2, NP], F32, name="psdw", tag="d")
                    psd_f = psd.rearrange("p s n -> p (s n)")
                    vf_f = v_fix[c].rearrange("p s n -> p (s n)")
                    # fp32r matmuls need even (8B-aligned) free offsets and
                    # counts. Odd-offset taps read a one-element-shifted copy
                    # (v_sh[q] = v[q-1]) and their windows are rounded inward
                    # to even bounds; the clipped columns only ever contribute
                    # halo zeros.
                    vsh = qkpool.tile([128, 2 * NP], F32R, name="vsh", tag="vsh")
                    nc.vector.tensor_copy(vsh[:, 1:2 * NP], vf_f[:, 0:2 * NP - 1])
                    for i, (dy, dx) in enumerate(taps):
                        off = 16 * dy + dx
                        n0, n1 = max(0, -off), 2 * NP - max(0, off)
                        t = (dy + 1) * 3 + (dx + 1)
                        if off % 2 == 0:
                            src, soff = vf_f, off
                        else:
                            n0, n1 = (n0 + 1) & ~1, n1 & ~1
                            src, soff = vsh, off + 1
                        nc.tensor.matmul(
                            psd_f[:, n0:n1],
                            dg_sb[:, c * 9 + t, :],
                            src[:, n0 + soff:n1 + soff],
                            start=(i == 0), stop=False,
                            skip_group_check=True,
                        )
                    for s in range(2):
                        ps = ps_at.tile([M0, 512], F32, name="psat", tag="a")
                        for m in range(2):
                            nc.tensor.matmul(
                                ps[:, m * NP:m * NP + NP],
                                kn_sb[kc][32 * hh:32 * hh + 32, s, m * M0:m * M0 + M0],
                                qn_sb[kc][32 * hh:32 * hh + 32, s, :],
                                start=True, stop=True,
                                tile_position=(32 * hh, 0),
                            )
                        src = ps.rearrange("p (m n) -> p m n", m=2)
                        if (s * NH + h) % 2 == 0:
                            nc.scalar.copy(attn_sb[h][:], src)
                        else:
                            nc.vector.tensor_copy(attn_sb[h][:], src)
                        for m in range(2):
                            nc.tensor.matmul(
                                psd[:, s, :],
                                vT_sb[(s, m)][:, c * 128:(c + 1) * 128],
                                attn_sb[c][:, m, :],
                                start=False, stop=(s == 1 and m == 1),
                                skip_group_check=True,
                            )
                    ot = opool.tile([128, 2, N], F32R, name="ot", tag=f"o{c}")
                    for ss in range(2):
                        nc.scalar.activation(
                            ot[:, ss, :].rearrange("p (y x) -> p y x", y=H),
                            psd[:, ss, :].rearrange("p (y x) -> p y x", y=16)[:, 1:15, 1:15],
                            mybir.ActivationFunctionType.Relu,
                            bias=bvl_sb[:, c:c + 1],
                        )
                    if c == 0:
                        out_sb.clear()
                    out_sb.append(ot)

                # ---- p conv + bias -> evac -> DMA out ----
                for oc in range(3):
                    ps = ps_main.tile([128, 392], F32, name="psp", tag="m")
                    for kc in range(8):
                        nc.tensor.matmul(
                            ps[:],
                            wp_sb[:, kc, oc * 128:(oc + 1) * 128],
                            out_sb[kc].rearrange("p s n -> p (s n)"),
                            start=(kc == 0), stop=False,
                        )
                    nc.tensor.matmul(
                        ps[:], bp_sb[:, oc * 128:(oc + 1) * 128], onesn_sb[:],
                        start=False, stop=True,
                    )
                    po = opool.tile([128, 2, N], F32, name="po", tag=f"po{oc}")
                    if oc % 2 == 0:
                        nc.vector.tensor_copy(po.rearrange("p s n -> p (s n)"), ps[:])
                    else:
                        nc.scalar.copy(po.rearrange("p s n -> p (s n)"), ps[:])
                    nc.sync.dma_start(
                        out=out_d[oc * 128:(oc + 1) * 128, 2 * p:2 * p + 2, :],
                        in_=po[:],
                    )

            if repeat == 1:
                for p in range(NPAIR):
                    pair_body(p)
            else:
                with tc.For_i(0, repeat):
                    for p in range(NPAIR):
                        pair_body(p)
        nc_lp.__exit__(None, None, None)

    _split_multi_waits(nc)
    return nc


def host_prep(inputs):
    def fold(w, b, gamma, beta, mean, var):
        s = gamma / np.sqrt(var + EPS)
        return (w * s.reshape(-1, *([1] * (w.ndim - 1)))).astype(np.float32), \
               ((b - mean) * s + beta).astype(np.float32)

    wq, bq = fold(inputs["wq"], inputs["bq"], inputs["q_gamma"], inputs["q_beta"],
                  inputs["q_mean"], inputs["q_var"])
    wk, bk = fold(inputs["wk"], inputs["bk"], inputs["k_gamma"], inputs["k_beta"],
                  inputs["k_mean"], inputs["k_var"])
    wv, bv = fold(inputs["wv"], inputs["bv"], inputs["v_gamma"], inputs["v_beta"],
                  inputs["v_mean"], inputs["v_var"])
    wvl, bvl = fold(inputs["wvl"], inputs["bvl"], inputs["vl_gamma"], inputs["vl_beta"],
                    inputs["vl_mean"], inputs["vl_var"])
    wp, bp = fold(inputs["wp"], inputs["bp"], inputs["p_gamma"], inputs["p_beta"],
                  inputs["p_mean"], inputs["p_var"])

    w9 = wvl.reshape(DH, 9)
    dgd = np.zeros((128, 72, 128), np.float32)
    pi = np.arange(128)
    for c in range(8):
        for t in range(9):
            dgd[pi, c * 9 + t, pi] = w9[c * 128 + pi, t]

    selq = np.zeros((8, 256), np.float32)
    for kc in range(2):
        for hh in range(4):
            selq[4 * kc + hh, kc * 128 + 32 * hh: kc * 128 + 32 * hh + 32] = 1.0
    # bo2[:, 16*qk + 8*kc : +8] is the lhsT for (q|k, chunk kc): column h
    # (head id) gets `val` at the partition rows of head h's kd-block within
    # the chunk; heads outside the chunk stay zero.
    bo2 = np.zeros((128, 32), np.float32)
    for qk in range(2):
        val = (1.0 / SCALE) if qk == 0 else 1.0
        for kc in range(2):
            for hh in range(4):
                h = 4 * kc + hh
                bo2[32 * hh:32 * hh + 32, 16 * qk + 8 * kc + h] = val

    return {
        "wqk": np.concatenate([wq, wk], 0).T.copy(),        # [384, 512]
        "wv": wv.T.copy(),                                  # [384, 1024]
        "wp": wp.T.copy(),                                  # [1024, 384]
        "dgd": dgd,
        "bqk": np.concatenate([bq, bk]).reshape(4, 128).T.copy(),
        "bv": bv.reshape(8, 128).T.copy(),
        "bvrow": bv.reshape(1, DH).copy(),
        "bvl": bvl.reshape(8, 128).T.copy(),
        "bp": bp.reshape(1, C).copy(),
        "selq": selq,
        "bo2": bo2,
        "ones1": np.ones((1, 128), np.float32),
        "onesn": np.ones((1, 392), np.float32),
    }


_NC_CACHE = {}


def kernel(**inputs):
    x = inputs["x"].astype(np.float32)
    shared = host_prep(inputs)
    xr = x.reshape(B, C, N)
    in_maps = []
    for c in range(NCORES):
        m = dict(shared)
        m["xin"] = np.ascontiguousarray(xr[c * S:(c + 1) * S].transpose(1, 0, 2))
        in_maps.append(m)

    if "nc" not in _NC_CACHE:
        _NC_CACHE["nc"] = build_nc()
    nc = _NC_CACHE["nc"]
    res = run_bass_kernel_spmd(nc, in_maps, list(range(NCORES)))
    outs = [res.results[c]["out"] for c in range(NCORES)]  # each [C, S, N]
    full = np.concatenate([o.transpose(1, 0, 2) for o in outs], axis=0)
    return full.reshape(B, C, H, H).astype(np.float32)
